# revision 17
# baseline (speedup 1.0000x reference)
"""Trainium2 Bass kernel for a 16-layer fully-connected chain (matvec per layer).

Computation (reference):
    v = x                       # [2048]
    for i in 0..13:  v = silu(W[i] @ v + b[i])
    out = W[14] @ v + b[14]

Strategy (8 NeuronCores):
  - Row-shard every layer: core c computes output neurons [c*256, (c+1)*256).
  - fp16 weights/activations with a per-layer power-of-4 rescaling so the
    growing activations (up to ~1e8) stay in fp16 range:
        vt_i  = v_i / 4^i        (the tensors that move through PE/collective)
        Wt_i  = W_i / 4 (i<14),  W_14 unscaled
        bt_i  = b_i / 4^(i+1)    (b_14 / 4^14)
    PSUM then holds  p = (W_i @ v_i + b_i) / 4^(i+1)  in fp32; the scalar
    engine computes v_true = silu(p * 4^(i+1)) and vt_{i+1} = v_true / 4^(i+1)
    (two activation ops).  Verified vs fp32 reference: rel err ~2e-3.
  - All 15 per-core weight slices (15 x 1 MB fp16) are resident in SBUF;
    they stream in up-front on the Sync-engine HWDGE queue with no
    dependencies, fully overlapped with the compute/collective chain.
  - The latency-critical small DMAs (activation bounce to DRAM for the
    AllGather, and the gathered-vector reload) go on the Scalar-engine
    HWDGE queue.  HWDGE completion counting-semaphores (DMAHW0..7) are
    assigned round-robin over ALL HWDGE dma_starts in issue order, so the
    python-level issue order below strictly alternates weight-class DMAs
    (even lanes) with small-class DMAs (odd lanes).  Without this, the
    collective trigger waits on a lane whose count includes a 1 MB weight
    prefetch, adding ~8 us of dead time per layer.
  - Per layer: bias matmul first (start=True, independent of v), then 16
    k-tile matmuls (lhsT = vt column, moving = 128x256 fp16 weight tile),
    silu+rescale on scalar, bounce DMA, AllGather (8 ranks), reload.
"""

import numpy as np

_L = 15        # number of weight matrices
_N = 2048      # neurons per layer
_M = 8         # cores
_SH = _N // _M  # 256 output slice per core
_KT = _N // 128  # 16 k-tiles

_CACHE = {}


def _build():
    import concourse.bacc as bacc
    import concourse.mybir as mybir
    import concourse.tile as tile

    f32 = mybir.dt.float32
    f16 = mybir.dt.float16
    AF = mybir.ActivationFunctionType

    nc = bacc.Bacc("TRN2", target_bir_lowering=False, debug=False,
                   num_devices=_M)
    # paces the weight prefetch to the compute: without it the 15 MB
    # up-front stream saturates HBM/DMA exactly when the first layers'
    # collectives and small DMAs need them (first AllGather stalled
    # ~40 us, layers 0-2 ran at 65/28/17 us vs the 14.5 us steady state)
    pace_sem = nc.alloc_semaphore("pace")

    wt = nc.dram_tensor("wt", [_L, 128, _KT * _SH], f16, kind="ExternalInput")
    # bias rows for all layers + a trailing constant 1.0 (rank-1 stationary
    # operand folding the bias add into the PSUM group)
    bias = nc.dram_tensor("bias", [1, _L * _SH + 1], f16, kind="ExternalInput")
    x0 = nc.dram_tensor("x0", [128, _KT], f16, kind="ExternalInput")
    out = nc.dram_tensor("out", [1, _SH], f32, kind="ExternalOutput")

    HK = _KT * _SH // 2  # half a layer's free size (2048 cols)

    with tile.TileContext(nc) as tc:
        with (
            tc.tile_pool(name="w", bufs=1) as wpool,
            tc.tile_pool(name="v", bufs=2) as vpool,
            tc.tile_pool(name="s", bufs=2) as spool,
            tc.tile_pool(name="consts", bufs=1) as cpool,
            tc.tile_pool(name="ps", bufs=4, space="PSUM") as pspool,
            tc.tile_pool(name="dram", bufs=2, space="DRAM") as dpool,
        ):
            # HWDGE lane parity: idx0 bias (even), idx1 x0 (odd), then the
            # loop contributes exactly 4 per iteration: w_a (even),
            # small (odd), w_b (even), small (odd).
            nc.gpsimd.sem_clear(pace_sem)

            # two warmup AllGathers: the first two collectives of a NEFF
            # pay ~40 us and ~15 us of ncfw/mesh cold-start (measured);
            # issuing tiny dummies at the head of the gpsimd queue absorbs
            # that under engine init + layer-0 compute.  The dummy input
            # is deliberately uninitialized: bypass-AllGather is a pure
            # byte copy (output unused), and any init DMA would either
            # delay the trigger (first-SWDGE-op init costs ~18 us) or
            # disturb the HWDGE lane parity.
            warm_in = dpool.tile([1, _KT], f16, tag="warmin")
            for _ in range(2):
                warm_out = dpool.tile([1, _KT * _M], f16, tag="warmout")
                nc.gpsimd.collective_compute(
                    "AllGather",
                    mybir.AluOpType.bypass,
                    replica_groups=[list(range(_M))],
                    ins=[warm_in.opt()],
                    outs=[warm_out.opt()],
                )

            bias_t = cpool.tile([1, _L * _SH + 1], f16)
            nc.sync.dma_start(bias_t[:], bias.ap())
            ones_t = bias_t[:, _L * _SH:_L * _SH + 1]

            v = vpool.tile([128, _KT], f16, tag="v")
            nc.sync.dma_start(v[:], x0.ap())

            scratch = cpool.tile([1, _KT], f16)
            scratch2 = cpool.tile([1, _KT], f16)

            w_tiles = []
            weight_waits = []
            s16_prev = None  # previous layer's fp16 activation slice
            for i in range(_L):
                w = wpool.tile([128, _KT * _SH], f16, tag=f"w{i}")
                w_tiles.append(w)
                # weight half A (even lane); from layer 3 on, gated (post-
                # Tile) on the bounce of layer i-3 so the stream paces the
                # compute instead of flooding the queues up front
                hw_a = nc.sync.dma_start(w[:, :HK], wt.ap()[i, :, :HK])
                if i >= 3:
                    weight_waits.append((hw_a, i - 2))
                # small DMA (odd lane): bounce of previous layer's act
                if i == 0:
                    nc.scalar.dma_start(scratch[:], x0.ap()[0:1, :])
                else:
                    cc_in = dpool.tile([1, _SH], f16, tag="ccin")
                    nc.scalar.dma_start(cc_in[:], s16_prev[:],
                                        single_packet=True)
                    # pace milestone: scalar queue, after the bounce, so it
                    # costs nothing on the critical path
                    nc.scalar.sem_inc(pace_sem, 1)
                # weight half B (even lane)
                nc.sync.dma_start(w[:, HK:], wt.ap()[i, :, HK:])
                # small DMAs (odd lanes), with an instant sync-queue dummy
                # in between to keep the even/odd lane parity: reload of
                # the gathered activations, split so the hi half's DMA +
                # semaphore latency hides under the first 8 k-tile matmuls
                if i == 0:
                    nc.scalar.dma_start(scratch[:], x0.ap()[0:1, :])
                    nc.sync.dma_start(scratch2[:], x0.ap()[0:1, :])
                    nc.scalar.dma_start(scratch[:], x0.ap()[0:1, :])
                else:
                    cc_out = dpool.tile([1, _N], f16, tag="ccout")
                    nc.gpsimd.collective_compute(
                        "AllGather",
                        mybir.AluOpType.bypass,
                        replica_groups=[list(range(_M))],
                        ins=[cc_in.opt()],
                        outs=[cc_out.opt()],
                    )
                    v = vpool.tile([128, _KT], f16, tag="v")
                    cc_v = cc_out[0, :].rearrange("(p t) -> p t", p=128)
                    nc.scalar.dma_start(v[:, 0:_KT // 2],
                                        cc_v[:, 0:_KT // 2],
                                        single_packet=True)
                    nc.sync.dma_start(scratch2[:], x0.ap()[0:1, :])
                    nc.scalar.dma_start(v[:, _KT // 2:],
                                        cc_v[:, _KT // 2:],
                                        single_packet=True)

                ps = pspool.tile([1, _SH], f32, tag="ps")
                # bias first: independent of v, runs during the AllGather
                nc.tensor.matmul(
                    ps[:],
                    lhsT=ones_t,
                    rhs=bias_t[:, i * _SH:(i + 1) * _SH],
                    start=True,
                    stop=False,
                )
                for t in range(_KT):
                    nc.tensor.matmul(
                        ps[:],
                        lhsT=v[:, t:t + 1],
                        rhs=w[:, t * _SH:(t + 1) * _SH],
                        start=False,
                        stop=(t == _KT - 1),
                    )

                if i < _L - 1:
                    # v_true = silu(p * 4^(i+1));  vt = v_true / 4^(i+1)
                    sc = float(4.0 ** (i + 1))
                    s32 = spool.tile([1, _SH], f32, tag="s32")
                    nc.scalar.activation(s32[:], ps[:], AF.Silu, scale=sc)
                    s16 = spool.tile([1, _SH], f16, tag="s16")
                    nc.scalar.activation(s16[:], s32[:], AF.Copy,
                                         scale=1.0 / sc)
                    s16_prev = s16
                else:
                    sout = spool.tile([1, _SH], f32, tag="sout")
                    nc.scalar.activation(sout[:], ps[:], AF.Copy,
                                         scale=float(4.0 ** 14))
                    nc.scalar.dma_start(out.ap(), sout[:],
                                        single_packet=True)

    # attached after Tile scheduling: the scheduler's single-core sim would
    # otherwise serialize everything behind these waits; post-hoc they only
    # delay the weight queue, which is always sound for counting sems
    for h, val in weight_waits:
        h.wait_op(pace_sem, val, "sem-ge", check=False)
    nc.compile()
    return nc


def _prep_inputs(x, W, b):
    """Host-side sharding/layout/scaling prep. k-index (p, t): k = p*16 + t."""
    W = np.asarray(W, dtype=np.float32)
    b = np.asarray(b, dtype=np.float32)
    x = np.asarray(x, dtype=np.float32)

    Ws = W.copy()
    Ws[:_L - 1] *= 0.25
    W16 = Ws.astype(np.float16)
    # W[i, m, k] with m = (c, j), k = (p, t)
    Wv = W16.reshape(_L, _M, _SH, 128, _KT)
    # -> [c, i, p, t, j]
    Wc = np.ascontiguousarray(Wv.transpose(1, 0, 3, 4, 2)).reshape(
        _M, _L, 128, _KT * _SH)

    scales = np.array([4.0 ** (i + 1) for i in range(_L - 1)] + [4.0 ** 14],
                      dtype=np.float32)
    bs = (b / scales[:, None]).astype(np.float16)

    x16 = np.ascontiguousarray(x.astype(np.float16).reshape(128, _KT))
    in_maps = []
    for c in range(_M):
        brow = np.concatenate([
            bs[:, c * _SH:(c + 1) * _SH].reshape(-1),
            np.ones(1, dtype=np.float16),
        ]).reshape(1, _L * _SH + 1)
        in_maps.append({
            "wt": np.ascontiguousarray(Wc[c]),
            "bias": np.ascontiguousarray(brow),
            "x0": x16,
        })
    return in_maps


def kernel(x, W, b, _trace=False):
    from concourse.bass_utils import run_bass_kernel_spmd

    key = "nc"
    if key not in _CACHE:
        _CACHE[key] = _build()
    nc = _CACHE[key]

    in_maps = _prep_inputs(x, W, b)
    res = run_bass_kernel_spmd(
        nc, in_maps, core_ids=list(range(_M)), trace=_trace)
    _CACHE["last_results"] = res
    return np.concatenate([res.results[c]["out"][0] for c in range(_M)])


# revision 18
# speedup vs baseline: 1.0431x; 1.0431x over previous
"""Trainium2 Bass kernel for a 16-layer fully-connected chain (matvec per layer).

Computation (reference):
    v = x                       # [2048]
    for i in 0..13:  v = silu(W[i] @ v + b[i])
    out = W[14] @ v + b[14]

Strategy (8 NeuronCores):
  - Row-shard every layer: core c computes output neurons [c*256, (c+1)*256).
  - fp16 weights/activations with a per-layer power-of-4 rescaling so the
    growing activations (up to ~1e8) stay in fp16 range:
        vt_i  = v_i / 4^i        (the tensors that move through PE/collective)
        Wt_i  = W_i / 4 (i<14),  W_14 unscaled
        bt_i  = b_i / 4^(i+1)    (b_14 / 4^14)
    PSUM then holds  p = (W_i @ v_i + b_i) / 4^(i+1)  in fp32; the scalar
    engine computes v_true = silu(p * 4^(i+1)) and vt_{i+1} = v_true / 4^(i+1)
    (two activation ops).  Verified vs fp32 reference: rel err ~2e-3.
  - All 15 per-core weight slices (15 x 1 MB fp16) are resident in SBUF;
    they stream in up-front on the Sync-engine HWDGE queue with no
    dependencies, fully overlapped with the compute/collective chain.
  - The latency-critical small DMAs (activation bounce to DRAM for the
    AllGather, and the gathered-vector reload) go on the Scalar-engine
    HWDGE queue.  HWDGE completion counting-semaphores (DMAHW0..7) are
    assigned round-robin over ALL HWDGE dma_starts in issue order, so the
    python-level issue order below strictly alternates weight-class DMAs
    (even lanes) with small-class DMAs (odd lanes).  Without this, the
    collective trigger waits on a lane whose count includes a 1 MB weight
    prefetch, adding ~8 us of dead time per layer.
  - Per layer: bias matmul first (start=True, independent of v), then 16
    k-tile matmuls (lhsT = vt column, moving = 128x256 fp16 weight tile),
    silu+rescale on scalar, bounce DMA, AllGather (8 ranks), reload.
"""

import numpy as np

_L = 15        # number of weight matrices
_N = 2048      # neurons per layer
_M = 8         # cores
_SH = _N // _M  # 256 output slice per core
_KT = _N // 128  # 16 k-tiles

_CACHE = {}


def _build():
    import concourse.bacc as bacc
    import concourse.mybir as mybir
    import concourse.tile as tile

    f32 = mybir.dt.float32
    f16 = mybir.dt.float16
    AF = mybir.ActivationFunctionType

    nc = bacc.Bacc("TRN2", target_bir_lowering=False, debug=False,
                   num_devices=_M)
    # paces the weight prefetch to the compute: without it the 15 MB
    # up-front stream saturates HBM/DMA exactly when the first layers'
    # collectives and small DMAs need them (first AllGather stalled
    # ~40 us, layers 0-2 ran at 65/28/17 us vs the 14.5 us steady state)
    pace_sem = nc.alloc_semaphore("pace")

    wt = nc.dram_tensor("wt", [_L, 128, _KT * _SH], f16, kind="ExternalInput")
    # bias rows for all layers + a trailing constant 1.0 (rank-1 stationary
    # operand folding the bias add into the PSUM group)
    bias = nc.dram_tensor("bias", [1, _L * _SH + 1], f16, kind="ExternalInput")
    x0 = nc.dram_tensor("x0", [128, _KT], f16, kind="ExternalInput")
    out = nc.dram_tensor("out", [1, _SH], f32, kind="ExternalOutput")

    HK = _KT * _SH // 2  # half a layer's free size (2048 cols)

    with tile.TileContext(nc) as tc:
        with (
            tc.tile_pool(name="w", bufs=1) as wpool,
            tc.tile_pool(name="v", bufs=2) as vpool,
            tc.tile_pool(name="s", bufs=2) as spool,
            tc.tile_pool(name="consts", bufs=1) as cpool,
            tc.tile_pool(name="ps", bufs=4, space="PSUM") as pspool,
            tc.tile_pool(name="dram", bufs=2, space="DRAM") as dpool,
        ):
            # HWDGE lane parity: idx0 bias (even), idx1 x0 (odd), then the
            # loop contributes exactly 4 per iteration: w_a (even),
            # small (odd), w_b (even), small (odd).
            nc.gpsimd.sem_clear(pace_sem)

            # two warmup AllGathers: the first two collectives of a NEFF
            # pay ~40 us and ~15 us of ncfw/mesh cold-start (measured);
            # issuing tiny dummies at the head of the gpsimd queue absorbs
            # the second entirely and overlaps part of the first.  The
            # input init rides the gpsimd SWDGE queue (first-SWDGE-op
            # init ~18 us, still ahead of the first real collective) so
            # the HWDGE lane parity is undisturbed; triggering with no
            # init at all measured slower (races NEFF init).
            warm_in = dpool.tile([1, _KT], f16, tag="warmin")
            nc.gpsimd.dma_start(warm_in[:], x0.ap()[0:1, :])
            for _ in range(2):
                warm_out = dpool.tile([1, _KT * _M], f16, tag="warmout")
                nc.gpsimd.collective_compute(
                    "AllGather",
                    mybir.AluOpType.bypass,
                    replica_groups=[list(range(_M))],
                    ins=[warm_in.opt()],
                    outs=[warm_out.opt()],
                )

            bias_t = cpool.tile([1, _L * _SH + 1], f16)
            nc.sync.dma_start(bias_t[:], bias.ap())
            ones_t = bias_t[:, _L * _SH:_L * _SH + 1]

            v = vpool.tile([128, _KT], f16, tag="v")
            nc.sync.dma_start(v[:], x0.ap())

            scratch = cpool.tile([1, _KT], f16)
            scratch2 = cpool.tile([1, _KT], f16)

            w_tiles = []
            weight_waits = []
            s16_prev = None  # previous layer's fp16 activation slice
            for i in range(_L):
                w = wpool.tile([128, _KT * _SH], f16, tag=f"w{i}")
                w_tiles.append(w)
                # weight half A (even lane); from layer 3 on, gated (post-
                # Tile) on the bounce of layer i-3 so the stream paces the
                # compute instead of flooding the queues up front
                hw_a = nc.sync.dma_start(w[:, :HK], wt.ap()[i, :, :HK])
                if i >= 3:
                    weight_waits.append((hw_a, i - 2))
                # small DMA (odd lane): bounce of previous layer's act
                if i == 0:
                    nc.scalar.dma_start(scratch[:], x0.ap()[0:1, :])
                else:
                    cc_in = dpool.tile([1, _SH], f16, tag="ccin")
                    nc.scalar.dma_start(cc_in[:], s16_prev[:],
                                        single_packet=True)
                    # pace milestone: scalar queue, after the bounce, so it
                    # costs nothing on the critical path
                    nc.scalar.sem_inc(pace_sem, 1)
                # weight half B (even lane)
                nc.sync.dma_start(w[:, HK:], wt.ap()[i, :, HK:])
                # small DMAs (odd lanes), with an instant sync-queue dummy
                # in between to keep the even/odd lane parity: reload of
                # the gathered activations, split so the hi half's DMA +
                # semaphore latency hides under the first 8 k-tile matmuls
                if i == 0:
                    nc.scalar.dma_start(scratch[:], x0.ap()[0:1, :])
                    nc.sync.dma_start(scratch2[:], x0.ap()[0:1, :])
                    nc.scalar.dma_start(scratch[:], x0.ap()[0:1, :])
                else:
                    cc_out = dpool.tile([1, _N], f16, tag="ccout")
                    nc.gpsimd.collective_compute(
                        "AllGather",
                        mybir.AluOpType.bypass,
                        replica_groups=[list(range(_M))],
                        ins=[cc_in.opt()],
                        outs=[cc_out.opt()],
                    )
                    v = vpool.tile([128, _KT], f16, tag="v")
                    cc_v = cc_out[0, :].rearrange("(p t) -> p t", p=128)
                    nc.scalar.dma_start(v[:, 0:_KT // 2],
                                        cc_v[:, 0:_KT // 2],
                                        single_packet=True)
                    nc.sync.dma_start(scratch2[:], x0.ap()[0:1, :])
                    nc.scalar.dma_start(v[:, _KT // 2:],
                                        cc_v[:, _KT // 2:],
                                        single_packet=True)

                ps = pspool.tile([1, _SH], f32, tag="ps")
                # bias first: independent of v, runs during the AllGather
                nc.tensor.matmul(
                    ps[:],
                    lhsT=ones_t,
                    rhs=bias_t[:, i * _SH:(i + 1) * _SH],
                    start=True,
                    stop=False,
                )
                for t in range(_KT):
                    nc.tensor.matmul(
                        ps[:],
                        lhsT=v[:, t:t + 1],
                        rhs=w[:, t * _SH:(t + 1) * _SH],
                        start=False,
                        stop=(t == _KT - 1),
                    )

                if i < _L - 1:
                    # v_true = silu(p * 4^(i+1));  vt = v_true / 4^(i+1)
                    sc = float(4.0 ** (i + 1))
                    s32 = spool.tile([1, _SH], f32, tag="s32")
                    nc.scalar.activation(s32[:], ps[:], AF.Silu, scale=sc)
                    s16 = spool.tile([1, _SH], f16, tag="s16")
                    nc.scalar.activation(s16[:], s32[:], AF.Copy,
                                         scale=1.0 / sc)
                    s16_prev = s16
                else:
                    sout = spool.tile([1, _SH], f32, tag="sout")
                    nc.scalar.activation(sout[:], ps[:], AF.Copy,
                                         scale=float(4.0 ** 14))
                    nc.scalar.dma_start(out.ap(), sout[:],
                                        single_packet=True)

    # attached after Tile scheduling: the scheduler's single-core sim would
    # otherwise serialize everything behind these waits; post-hoc they only
    # delay the weight queue, which is always sound for counting sems
    for h, val in weight_waits:
        h.wait_op(pace_sem, val, "sem-ge", check=False)
    nc.compile()
    return nc


def _prep_inputs(x, W, b):
    """Host-side sharding/layout/scaling prep. k-index (p, t): k = p*16 + t."""
    W = np.asarray(W, dtype=np.float32)
    b = np.asarray(b, dtype=np.float32)
    x = np.asarray(x, dtype=np.float32)

    Ws = W.copy()
    Ws[:_L - 1] *= 0.25
    W16 = Ws.astype(np.float16)
    # W[i, m, k] with m = (c, j), k = (p, t)
    Wv = W16.reshape(_L, _M, _SH, 128, _KT)
    # -> [c, i, p, t, j]
    Wc = np.ascontiguousarray(Wv.transpose(1, 0, 3, 4, 2)).reshape(
        _M, _L, 128, _KT * _SH)

    scales = np.array([4.0 ** (i + 1) for i in range(_L - 1)] + [4.0 ** 14],
                      dtype=np.float32)
    bs = (b / scales[:, None]).astype(np.float16)

    x16 = np.ascontiguousarray(x.astype(np.float16).reshape(128, _KT))
    in_maps = []
    for c in range(_M):
        brow = np.concatenate([
            bs[:, c * _SH:(c + 1) * _SH].reshape(-1),
            np.ones(1, dtype=np.float16),
        ]).reshape(1, _L * _SH + 1)
        in_maps.append({
            "wt": np.ascontiguousarray(Wc[c]),
            "bias": np.ascontiguousarray(brow),
            "x0": x16,
        })
    return in_maps


def kernel(x, W, b, _trace=False):
    from concourse.bass_utils import run_bass_kernel_spmd

    key = "nc"
    if key not in _CACHE:
        _CACHE[key] = _build()
    nc = _CACHE[key]

    in_maps = _prep_inputs(x, W, b)
    res = run_bass_kernel_spmd(
        nc, in_maps, core_ids=list(range(_M)), trace=_trace)
    _CACHE["last_results"] = res
    return np.concatenate([res.results[c]["out"][0] for c in range(_M)])
